# revision 3
# baseline (speedup 1.0000x reference)
"""Trainium2 Bass kernel v3: causal cosine-sim attention (nn_Attention_33930241638513).

Shapes: x [1, 4096, 1024], Wq/Wk/Wv/Wo [1024, 1024], 16 heads, dh=64, scale=8.0.

Sharding (8 cores): 2 heads per core. Wq/Wk/Wv column-sharded (128 cols/core),
Wo row-sharded (128 rows/core). Each core computes its 2 heads end-to-end and a
partial [4096, 1024] output; host sums the 8 partials.

v3 structure (software-pipelined, chunk lag 1):
  for cb: P(cb) projections -> N(cb) norms -> A(cb-1) attention + tail(cb-2)
  - bf16 operand pipeline, fp32 PSUM accumulation.
  - x loaded with casting gpsimd DMA (f32 HBM -> bf16 SBUF); transposes run
    1 cycle/row on PE instead of 1.5.
  - q,k normalized explicitly per chunk; Ln/Ln then Exp/Exp grouped so the
    activation table swaps twice per chunk, not four times. The 8.0 softmax
    scale rides the k-broadcast matrix (selMb8).
  - Attention exp fused across both heads ([128, 2, 512] 2-bank PSUM tile).
  - Causal diagonal mask via gpsimd affine_select on the idle Pool engine.
"""

import math
import sys

import numpy as np

sys.path.insert(0, "/opt/trn_rl_repo")

import concourse.bacc as bacc  # noqa: E402
import concourse.mybir as mybir  # noqa: E402
from concourse.bass_utils import run_bass_kernel_spmd  # noqa: E402
from concourse.masks import make_identity  # noqa: E402
from concourse.tile import TileContext  # noqa: E402

F32 = mybir.dt.float32
F32R = mybir.dt.float32r
BF16 = mybir.dt.bfloat16
AF = mybir.ActivationFunctionType

N = 4096
D = 1024
C = 128
DH = 64
NCORES = 8
NCHUNK = 8
CH = 512

LAST_EXEC_NS = None


def build_nc(reps=1):
    # reps>1 unrolls the whole body N times (same I/O); test.py differences
    # reps=4 vs reps=1 chain slopes to isolate pure device time.
    nc = bacc.Bacc(None, target_bir_lowering=False, debug=False)
    x_d = nc.dram_tensor("x", [N, D], F32, kind="ExternalInput")
    wq_d = nc.dram_tensor("wq", [D, C], F32, kind="ExternalInput")
    wk_d = nc.dram_tensor("wk", [D, C], F32, kind="ExternalInput")
    wv_d = nc.dram_tensor("wv", [D, C], F32, kind="ExternalInput")
    wo_d = nc.dram_tensor("wo", [C, D], F32, kind="ExternalInput")
    out_d = nc.dram_tensor("out", [N, D], F32, kind="ExternalOutput")

    with TileContext(nc) as tc:
        with (
            tc.tile_pool(name="const", bufs=1) as cpool,
            tc.tile_pool(name="big", bufs=1) as bpool,
        ):
            # ---------------- constants ----------------
            # Pre-load the ln+exp shared activation table set so the
            # table-load pass never needs to swap between Ln and Exp.
            nc.scalar.add_instruction(
                mybir.InstLoadActFuncSet(
                    name=f"I-{nc.next_id()}", ins=[], outs=[], act_func_set_id=6
                )
            )
            ident = cpool.tile([128, 128], F32, tag="ident")
            make_identity(nc, ident)
            identB = cpool.tile([128, 128], BF16, tag="identB")
            nc.vector.tensor_copy(identB, ident)

            onesM = cpool.tile([128, 65], F32, tag="onesM")
            nc.gpsimd.memset(onesM, 1.0)
            nc.gpsimd.memset(onesM[64:128, 0:1], 0.0)
            nc.gpsimd.memset(onesM[0:64, 64:65], 0.0)
            onesMb = cpool.tile([128, 65], BF16, tag="onesMb")
            nc.vector.tensor_copy(onesMb, onesM)

            # selM: broadcast rows 0/64 of a [65, n] tile to 128 partitions;
            # selM8 bakes the 8.0 softmax scale into the k broadcast.
            selM = cpool.tile([65, 128], F32, tag="selM")
            nc.gpsimd.memset(selM, 0.0)
            nc.gpsimd.memset(selM[0:1, 0:64], 1.0)
            nc.gpsimd.memset(selM[64:65, 64:128], 1.0)
            selMb = cpool.tile([65, 128], BF16, tag="selMb")
            nc.vector.tensor_copy(selMb, selM)
            selM8 = cpool.tile([65, 128], F32, tag="selM8")
            nc.gpsimd.memset(selM8, 0.0)
            nc.gpsimd.memset(selM8[0:1, 0:64], 8.0)
            nc.gpsimd.memset(selM8[64:65, 64:128], 8.0)
            selMb8 = cpool.tile([65, 128], BF16, tag="selMb8")
            nc.vector.tensor_copy(selMb8, selM8)

            ones32 = cpool.tile([128, 32], BF16, tag="ones32")
            nc.gpsimd.memset(ones32, 1.0)

            # weights: DMA f32 staging (scoped), DVE-cast to bf16
            wq_sb = cpool.tile([128, 8, C], BF16, tag="wq")
            wk_sb = cpool.tile([128, 8, C], BF16, tag="wk")
            wv_sb = cpool.tile([128, 8, C], BF16, tag="wv")
            wo_sb = cpool.tile([128, D], BF16, tag="wo")
            with tc.tile_pool(name="wstage", bufs=1) as wstage:
                for wd, wsb, rearr in (
                    (wq_d, wq_sb, True),
                    (wk_d, wk_sb, True),
                    (wv_d, wv_sb, True),
                    (wo_d, wo_sb, False),
                ):
                    wf = wstage.tile([128, 8, C], F32, tag="wf")
                    if rearr:
                        nc.sync.dma_start(wf, wd.rearrange("(dc p) c -> p dc c", p=128))
                        nc.vector.tensor_copy(
                            wsb.rearrange("p a c -> p (a c)"),
                            wf.rearrange("p a c -> p (a c)"),
                        )
                    else:
                        nc.sync.dma_start(wf.rearrange("p a c -> p (a c)"), wd[:, :])
                        nc.vector.tensor_copy(wsb, wf.rearrange("p a c -> p (a c)"))

            # ---------------- persistent big buffers ----------------
            qT = bpool.tile([128, N], BF16, tag="qT")
            kT = bpool.tile([128, N], BF16, tag="kT")  # holds 8 * k-hat
            v_all = bpool.tile([128, 32, 129], BF16, tag="v_all")
            nc.vector.tensor_copy(
                v_all[:, :, 64:65].rearrange("p a b -> p (a b)"), ones32
            )
            oT = bpool.tile([128, N], BF16, tag="oT")
            l_row = bpool.tile([65, N], BF16, tag="l_row")
            nc.gpsimd.memset(l_row, 1.0)

            def body(rep_tag):
                with (
                    tc.tile_pool(name=f"xnat{rep_tag}", bufs=4) as xnat_pool,
                    tc.tile_pool(name=f"xt{rep_tag}", bufs=2) as xt_pool,
                    tc.tile_pool(name=f"wrk{rep_tag}", bufs=2) as wrk_pool,
                    tc.tile_pool(name=f"p{rep_tag}", bufs=2) as p_pool,
                    tc.tile_pool(name=f"psw{rep_tag}", bufs=2, space="PSUM") as psw,
                    tc.tile_pool(name=f"psst{rep_tag}", bufs=2, space="PSUM") as psst,
                    tc.tile_pool(name=f"psot{rep_tag}", bufs=1, space="PSUM") as psot,
                ):
                    def phase_P(cb):
                        n0 = cb * CH
                        xn = []
                        for nb in range(4):
                            t = xnat_pool.tile([128, D], BF16, tag="xn")
                            nc.gpsimd.dma_start(
                                t, x_d[n0 + nb * 128 : n0 + (nb + 1) * 128, :]
                            )
                            xn.append(t)
                        xt = xt_pool.tile([128, 8, CH], BF16, tag="xt")
                        for dc in range(8):
                            tp = psw.tile([128, CH], BF16, tag="w", name=f"tp{rep_tag}")
                            for nb in range(4):
                                nc.tensor.transpose(
                                    tp[:, nb * 128 : (nb + 1) * 128],
                                    xn[nb][:, dc * 128 : (dc + 1) * 128],
                                    identB,
                                )
                            nc.vector.tensor_copy(xt[:, dc, :], tp)

                        raws = {}
                        ssqs = {}
                        for kind, w_sb in (("q", wq_sb), ("k", wk_sb), ("v", wv_sb)):
                            acc = psw.tile(
                                [128, CH], F32, tag="w", name=f"acc{kind}{rep_tag}"
                            )
                            for dc in range(8):
                                nc.tensor.matmul(
                                    acc,
                                    lhsT=w_sb[:, dc, :],
                                    rhs=xt[:, dc, :],
                                    start=(dc == 0),
                                    stop=(dc == 7),
                                )
                            if kind in ("q", "k"):
                                raw = wrk_pool.tile(
                                    [128, CH], BF16, tag=f"raw{kind}"
                                )
                                nc.vector.tensor_copy(raw, acc)
                                sq = wrk_pool.tile([128, CH], BF16, tag="sq")
                                nc.vector.tensor_mul(sq, raw, raw)
                                ssq = psw.tile(
                                    [65, CH], F32, tag="w", name=f"ssq{kind}{rep_tag}"
                                )
                                nc.tensor.matmul(
                                    ssq, lhsT=onesMb, rhs=sq, start=True, stop=True
                                )
                                raws[kind] = raw
                                ssqs[kind] = ssq
                            else:
                                vtmp = wrk_pool.tile([128, CH], BF16, tag="vtmp")
                                nc.vector.tensor_copy(vtmp, acc)
                                vn = psw.tile(
                                    [128, CH], BF16, tag="w", name=f"vn{rep_tag}"
                                )
                                for nb in range(4):
                                    nc.tensor.transpose(
                                        vn[:, nb * 128 : (nb + 1) * 128],
                                        vtmp[:, nb * 128 : (nb + 1) * 128],
                                        identB,
                                    )
                                for nb in range(4):
                                    jb = cb * 4 + nb
                                    nc.vector.tensor_copy(
                                        v_all[:, jb, 0:64],
                                        vn[:, nb * 128 : nb * 128 + 64],
                                    )
                                    nc.vector.tensor_copy(
                                        v_all[:, jb, 65:129],
                                        vn[:, nb * 128 + 64 : (nb + 1) * 128],
                                    )
                        return raws, ssqs

                    def phase_N(cb, raws, ssqs):
                        n0 = cb * CH
                        # Ln, Ln then Exp, Exp: two table swaps per chunk
                        lg = wrk_pool.tile([65, 2, CH], F32, tag="lg")
                        nc.scalar.activation(lg[:, 0, :], ssqs["q"], AF.Ln)
                        nc.scalar.activation(lg[:, 1, :], ssqs["k"], AF.Ln)
                        inv = wrk_pool.tile([65, 2, CH], BF16, tag="inv")
                        nc.scalar.activation(inv[:, 0, :], lg[:, 0, :], AF.Exp, scale=-0.5)
                        nc.scalar.activation(inv[:, 1, :], lg[:, 1, :], AF.Exp, scale=-0.5)
                        for i, (kind, sel, dst) in enumerate(
                            (("q", selMb, qT), ("k", selMb8, kT))
                        ):
                            bc = psw.tile(
                                [128, CH], F32, tag="w", name=f"bc{kind}{rep_tag}"
                            )
                            nc.tensor.matmul(
                                bc, lhsT=sel, rhs=inv[:, i, :], start=True, stop=True
                            )
                            nc.vector.tensor_mul(
                                dst[:, n0 : n0 + CH], raws[kind], bc
                            )

                    def emit_tail(bi):
                        i0 = bi * CH
                        lps = psw.tile([128, CH], F32, tag="w", name=f"lps{rep_tag}")
                        nc.tensor.matmul(
                            lps,
                            lhsT=selMb,
                            rhs=l_row[:, i0 : i0 + CH],
                            start=True,
                            stop=True,
                        )
                        rbc = wrk_pool.tile([128, CH], F32, tag="rbc")
                        nc.vector.reciprocal(rbc, lps)
                        nc.vector.tensor_mul(
                            oT[:, i0 : i0 + CH], oT[:, i0 : i0 + CH], rbc
                        )
                        for ic in range(4 * bi, 4 * (bi + 1)):
                            osb = wrk_pool.tile([128, D], F32, tag="osb")
                            for nh in range(2):
                                op = psw.tile(
                                    [128, CH], F32, tag="w", name=f"op{rep_tag}"
                                )
                                nc.tensor.matmul(
                                    op,
                                    lhsT=oT[:, ic * 128 : (ic + 1) * 128],
                                    rhs=wo_sb[:, nh * CH : (nh + 1) * CH],
                                    start=True,
                                    stop=True,
                                )
                                nc.vector.tensor_copy(
                                    osb[:, nh * CH : (nh + 1) * CH], op
                                )
                            nc.sync.dma_start(out_d[ic * 128 : (ic + 1) * 128, :], osb)

                    def phase_A(bi):
                        i0 = bi * CH
                        njb = 4 * (bi + 1)
                        ot = psot.tile([65, 2, CH], F32, tag="ot")
                        for jb in range(njb):
                            first = jb == 0
                            last = jb == njb - 1
                            # diagonal blocks (t>0) only cover i >= jb*128:
                            # shrink the i-window instead of masking it all
                            t = jb - 4 * bi
                            ioff = 128 * t if t > 0 else 0
                            w = CH - ioff
                            st = psst.tile(
                                [128, 2, CH], F32, tag="st", name=f"st{rep_tag}"
                            )
                            for h in range(2):
                                nc.tensor.matmul(
                                    st[:, h, 0:w],
                                    lhsT=kT[
                                        64 * h : 64 * (h + 1),
                                        jb * 128 : (jb + 1) * 128,
                                    ],
                                    rhs=qT[
                                        64 * h : 64 * (h + 1),
                                        i0 + ioff : i0 + CH,
                                    ],
                                    start=True,
                                    stop=True,
                                    tile_position=(64 * h, 0),
                                )
                            p = p_pool.tile([128, 2, CH], BF16, tag="p")
                            nc.scalar.activation(p[:, :, 0:w], st[:, :, 0:w], AF.Exp)
                            if t >= 0:
                                # keep where i-offset f >= partition j-offset
                                nc.gpsimd.affine_select(
                                    out=p[:, :, 0:w],
                                    in_=p[:, :, 0:w],
                                    compare_op=mybir.AluOpType.is_ge,
                                    fill=0.0,
                                    base=0,
                                    channel_multiplier=-1,
                                    pattern=[[0, 2], [1, w]],
                                )
                            nc.tensor.matmul(
                                ot[:, 0, ioff:CH],
                                lhsT=v_all[:, jb, 0:65],
                                rhs=p[:, 0, 0:w],
                                start=first,
                                stop=last,
                            )
                            nc.tensor.matmul(
                                ot[:, 1, ioff:CH],
                                lhsT=v_all[:, jb, 64:129],
                                rhs=p[:, 1, 0:w],
                                start=first,
                                stop=last,
                            )
                        stg = []
                        for h in range(2):
                            s = wrk_pool.tile(
                                [65, CH], BF16, tag="ostg", name=f"ostg{h}{rep_tag}"
                            )
                            nc.vector.tensor_copy(s, ot[:, h, :])
                            stg.append(s)
                        nc.sync.dma_start(oT[0:64, i0 : i0 + CH], stg[0][0:64, :])
                        nc.sync.dma_start(oT[64:128, i0 : i0 + CH], stg[1][1:65, :])
                        nc.sync.dma_start(l_row[0:1, i0 : i0 + CH], stg[0][64:65, :])
                        nc.sync.dma_start(l_row[64:65, i0 : i0 + CH], stg[1][0:1, :])

                    for cb in range(NCHUNK):
                        raws, ssqs = phase_P(cb)
                        phase_N(cb, raws, ssqs)
                        if cb >= 1:
                            phase_A(cb - 1)
                            if cb >= 2:
                                emit_tail(cb - 2)
                    emit_tail(NCHUNK - 2)
                    phase_A(NCHUNK - 1)
                    emit_tail(NCHUNK - 1)

            for _r in range(reps):
                body("" if _r == 0 else f"r{_r}")
    nc.compile()
    return nc


def _in_maps(x, Wq, Wk, Wv, Wo):
    maps = []
    for c in range(NCORES):
        cs = slice(c * C, (c + 1) * C)
        maps.append(
            {
                "x": x,
                "wq": np.ascontiguousarray(Wq[:, cs]),
                "wk": np.ascontiguousarray(Wk[:, cs]),
                "wv": np.ascontiguousarray(Wv[:, cs]),
                "wo": np.ascontiguousarray(Wo[cs, :]),
            }
        )
    return maps


_CACHE = {}


def _run_cached(in_maps):
    """Compile once per process; later kernel() calls only dispatch."""
    import jax
    from jax.experimental.shard_map import shard_map
    from jax.sharding import Mesh, PartitionSpec

    from concourse import bass2jax

    if "fn" not in _CACHE:
        bass2jax.install_neuronx_cc_hook()
        nc = build_nc()
        partition_name = (
            nc.partition_id_tensor.name if nc.partition_id_tensor else None
        )
        in_names, out_names, out_avals, zero_outs = [], [], [], []
        for alloc in nc.m.functions[0].allocations:
            if not isinstance(alloc, mybir.MemoryLocationSet):
                continue
            name = alloc.memorylocations[0].name
            if alloc.kind == "ExternalInput":
                if name != partition_name:
                    in_names.append(name)
            elif alloc.kind == "ExternalOutput":
                shape = list(alloc.tensor_shape)
                npdt = mybir.dt.np(alloc.dtype)
                out_names.append(name)
                out_avals.append(jax.core.ShapedArray(shape, npdt))
                zero_outs.append(np.zeros(shape, npdt))
        n_params = len(in_names)
        n_outs = len(out_avals)
        all_names = in_names + out_names
        if partition_name is not None:
            all_names.append(partition_name)
        donate = tuple(range(n_params, n_params + n_outs))

        def _body(*args):
            operands = list(args)
            if partition_name is not None:
                operands.append(bass2jax.partition_id_tensor())
            outs = bass2jax._bass_exec_p.bind(
                *operands,
                out_avals=tuple(out_avals),
                in_names=tuple(all_names),
                out_names=tuple(out_names),
                lowering_input_output_aliases=(),
                sim_require_finite=True,
                sim_require_nnan=True,
                nc=nc,
            )
            return tuple(outs)

        devices = jax.devices()[:NCORES]
        mesh = Mesh(np.asarray(devices), ("core",))
        fn = jax.jit(
            shard_map(
                _body,
                mesh=mesh,
                in_specs=(PartitionSpec("core"),) * (n_params + n_outs),
                out_specs=(PartitionSpec("core"),) * n_outs,
                check_rep=False,
            ),
            donate_argnums=donate,
            keep_unused=True,
        )
        _CACHE.update(
            fn=fn,
            in_names=in_names,
            out_names=out_names,
            zero_outs=zero_outs,
            mesh=mesh,
        )

    fn = _CACHE["fn"]
    mesh = _CACHE["mesh"]
    import jax as _jax
    from jax.sharding import NamedSharding, PartitionSpec as _P

    sh = NamedSharding(mesh, _P("core"))
    per_core = [
        [np.asarray(m[nm]) for nm in _CACHE["in_names"]] for m in in_maps
    ]
    concat_in = [
        np.concatenate([per_core[c][i] for c in range(NCORES)], axis=0)
        for i in range(len(_CACHE["in_names"]))
    ]
    dev_in = [_jax.device_put(a, sh) for a in concat_in]
    dz = [
        _jax.device_put(
            np.zeros((NCORES * z.shape[0], *z.shape[1:]), z.dtype), sh
        )
        for z in _CACHE["zero_outs"]
    ]
    outs = fn(*dev_in, *dz)
    res = []
    for c in range(NCORES):
        m = {}
        for i, nm in enumerate(_CACHE["out_names"]):
            rows = _CACHE["zero_outs"][i].shape[0]
            m[nm] = np.asarray(outs[i][c * rows : (c + 1) * rows])
        res.append(m)
    return res


def kernel(x, Wq, Wk, Wv, Wo):
    global LAST_EXEC_NS
    x = np.ascontiguousarray(np.asarray(x, dtype=np.float32).reshape(N, D))
    Wq = np.asarray(Wq, dtype=np.float32)
    Wk = np.asarray(Wk, dtype=np.float32)
    Wv = np.asarray(Wv, dtype=np.float32)
    Wo = np.asarray(Wo, dtype=np.float32)

    in_maps = _in_maps(x, Wq, Wk, Wv, Wo)
    try:
        results = _run_cached(in_maps)
    except Exception:
        nc = build_nc()
        res = run_bass_kernel_spmd(nc, in_maps, core_ids=list(range(NCORES)))
        LAST_EXEC_NS = getattr(res, "exec_time_ns", None)
        results = res.results

    out = np.zeros((N, D), dtype=np.float32)
    for c in range(NCORES):
        out += results[c]["out"]
    return out.reshape(1, N, D)


# revision 4
# speedup vs baseline: 1.0042x; 1.0042x over previous
"""Trainium2 Bass kernel v3: causal cosine-sim attention (nn_Attention_33930241638513).

Shapes: x [1, 4096, 1024], Wq/Wk/Wv/Wo [1024, 1024], 16 heads, dh=64, scale=8.0.

Sharding (8 cores): 2 heads per core. Wq/Wk/Wv column-sharded (128 cols/core),
Wo row-sharded (128 rows/core). Each core computes its 2 heads end-to-end and a
partial [4096, 1024] output; host sums the 8 partials.

v3 structure (software-pipelined, chunk lag 1):
  for cb: P(cb) projections -> N(cb) norms -> A(cb-1) attention + tail(cb-2)
  - bf16 operand pipeline, fp32 PSUM accumulation.
  - x loaded with casting gpsimd DMA (f32 HBM -> bf16 SBUF); transposes run
    1 cycle/row on PE instead of 1.5.
  - q,k normalized explicitly per chunk; Ln/Ln then Exp/Exp grouped so the
    activation table swaps twice per chunk, not four times. The 8.0 softmax
    scale rides the k-broadcast matrix (selMb8).
  - Attention exp fused across both heads ([128, 2, 512] 2-bank PSUM tile).
  - Causal diagonal mask via gpsimd affine_select on the idle Pool engine.
"""

import math
import sys

import numpy as np

sys.path.insert(0, "/opt/trn_rl_repo")

import concourse.bacc as bacc  # noqa: E402
import concourse.mybir as mybir  # noqa: E402
from concourse.bass_utils import run_bass_kernel_spmd  # noqa: E402
from concourse.masks import make_identity  # noqa: E402
from concourse.tile import TileContext  # noqa: E402

F32 = mybir.dt.float32
F32R = mybir.dt.float32r
BF16 = mybir.dt.bfloat16
AF = mybir.ActivationFunctionType

N = 4096
D = 1024
C = 128
DH = 64
NCORES = 8
NCHUNK = 8
CH = 512

LAST_EXEC_NS = None


def build_nc(reps=1):
    # reps>1 unrolls the whole body N times (same I/O); test.py differences
    # reps=4 vs reps=1 chain slopes to isolate pure device time.
    nc = bacc.Bacc(None, target_bir_lowering=False, debug=False)
    x_d = nc.dram_tensor("x", [N, D], F32, kind="ExternalInput")
    wq_d = nc.dram_tensor("wq", [D, C], F32, kind="ExternalInput")
    wk_d = nc.dram_tensor("wk", [D, C], F32, kind="ExternalInput")
    wv_d = nc.dram_tensor("wv", [D, C], F32, kind="ExternalInput")
    wo_d = nc.dram_tensor("wo", [C, D], F32, kind="ExternalInput")
    out_d = nc.dram_tensor("out", [N, D], F32, kind="ExternalOutput")

    with TileContext(nc) as tc:
        with (
            tc.tile_pool(name="const", bufs=1) as cpool,
            tc.tile_pool(name="big", bufs=1) as bpool,
        ):
            # ---------------- constants ----------------
            # Pre-load the ln+exp shared activation table set so the
            # table-load pass never needs to swap between Ln and Exp.
            nc.scalar.add_instruction(
                mybir.InstLoadActFuncSet(
                    name=f"I-{nc.next_id()}", ins=[], outs=[], act_func_set_id=6
                )
            )
            ident = cpool.tile([128, 128], F32, tag="ident")
            make_identity(nc, ident)
            identB = cpool.tile([128, 128], BF16, tag="identB")
            nc.vector.tensor_copy(identB, ident)

            onesM = cpool.tile([128, 65], F32, tag="onesM")
            nc.gpsimd.memset(onesM, 1.0)
            nc.gpsimd.memset(onesM[64:128, 0:1], 0.0)
            nc.gpsimd.memset(onesM[0:64, 64:65], 0.0)
            onesMb = cpool.tile([128, 65], BF16, tag="onesMb")
            nc.vector.tensor_copy(onesMb, onesM)

            # selM: broadcast rows 0/64 of a [65, n] tile to 128 partitions;
            # selM8 bakes the 8.0 softmax scale into the k broadcast.
            selM = cpool.tile([65, 128], F32, tag="selM")
            nc.gpsimd.memset(selM, 0.0)
            nc.gpsimd.memset(selM[0:1, 0:64], 1.0)
            nc.gpsimd.memset(selM[64:65, 64:128], 1.0)
            selMb = cpool.tile([65, 128], BF16, tag="selMb")
            nc.vector.tensor_copy(selMb, selM)
            selM8 = cpool.tile([65, 128], F32, tag="selM8")
            nc.gpsimd.memset(selM8, 0.0)
            nc.gpsimd.memset(selM8[0:1, 0:64], 8.0)
            nc.gpsimd.memset(selM8[64:65, 64:128], 8.0)
            selMb8 = cpool.tile([65, 128], BF16, tag="selMb8")
            nc.vector.tensor_copy(selMb8, selM8)

            ones32 = cpool.tile([128, 32], BF16, tag="ones32")
            nc.gpsimd.memset(ones32, 1.0)

            # weights: DMA f32 staging (scoped), DVE-cast to bf16
            wq_sb = cpool.tile([128, 8, C], BF16, tag="wq")
            wk_sb = cpool.tile([128, 8, C], BF16, tag="wk")
            wv_sb = cpool.tile([128, 8, C], BF16, tag="wv")
            wo_sb = cpool.tile([128, D], BF16, tag="wo")
            with tc.tile_pool(name="wstage", bufs=1) as wstage:
                for wd, wsb, rearr in (
                    (wq_d, wq_sb, True),
                    (wk_d, wk_sb, True),
                    (wv_d, wv_sb, True),
                    (wo_d, wo_sb, False),
                ):
                    wf = wstage.tile([128, 8, C], F32, tag="wf")
                    if rearr:
                        nc.sync.dma_start(wf, wd.rearrange("(dc p) c -> p dc c", p=128))
                        nc.vector.tensor_copy(
                            wsb.rearrange("p a c -> p (a c)"),
                            wf.rearrange("p a c -> p (a c)"),
                        )
                    else:
                        nc.sync.dma_start(wf.rearrange("p a c -> p (a c)"), wd[:, :])
                        nc.vector.tensor_copy(wsb, wf.rearrange("p a c -> p (a c)"))

            # ---------------- persistent big buffers ----------------
            qT = bpool.tile([128, N], BF16, tag="qT")
            kT = bpool.tile([128, N], BF16, tag="kT")  # holds 8 * k-hat
            v_all = bpool.tile([128, 32, 129], BF16, tag="v_all")
            nc.vector.tensor_copy(
                v_all[:, :, 64:65].rearrange("p a b -> p (a b)"), ones32
            )
            oT = bpool.tile([128, N], BF16, tag="oT")
            l_row = bpool.tile([65, N], BF16, tag="l_row")
            nc.gpsimd.memset(l_row, 1.0)

            def body(rep_tag):
                with (
                    tc.tile_pool(name=f"xnat{rep_tag}", bufs=4) as xnat_pool,
                    tc.tile_pool(name=f"xt{rep_tag}", bufs=2) as xt_pool,
                    tc.tile_pool(name=f"wrk{rep_tag}", bufs=2) as wrk_pool,
                    tc.tile_pool(name=f"p{rep_tag}", bufs=3) as p_pool,
                    tc.tile_pool(name=f"osb{rep_tag}", bufs=1) as osb_pool,
                    tc.tile_pool(name=f"psw{rep_tag}", bufs=2, space="PSUM") as psw,
                    tc.tile_pool(name=f"psst{rep_tag}", bufs=2, space="PSUM") as psst,
                    tc.tile_pool(name=f"psot{rep_tag}", bufs=1, space="PSUM") as psot,
                ):
                    def phase_P(cb):
                        n0 = cb * CH
                        xn = []
                        for nb in range(4):
                            t = xnat_pool.tile([128, D], BF16, tag="xn")
                            nc.gpsimd.dma_start(
                                t, x_d[n0 + nb * 128 : n0 + (nb + 1) * 128, :]
                            )
                            xn.append(t)
                        xt = xt_pool.tile([128, 8, CH], BF16, tag="xt")
                        for dc in range(8):
                            tp = psw.tile([128, CH], BF16, tag="w", name=f"tp{rep_tag}")
                            for nb in range(4):
                                nc.tensor.transpose(
                                    tp[:, nb * 128 : (nb + 1) * 128],
                                    xn[nb][:, dc * 128 : (dc + 1) * 128],
                                    identB,
                                )
                            nc.vector.tensor_copy(xt[:, dc, :], tp)

                        raws = {}
                        ssqs = {}
                        for kind, w_sb in (("q", wq_sb), ("k", wk_sb), ("v", wv_sb)):
                            acc = psw.tile(
                                [128, CH], F32, tag="w", name=f"acc{kind}{rep_tag}"
                            )
                            for dc in range(8):
                                nc.tensor.matmul(
                                    acc,
                                    lhsT=w_sb[:, dc, :],
                                    rhs=xt[:, dc, :],
                                    start=(dc == 0),
                                    stop=(dc == 7),
                                )
                            if kind in ("q", "k"):
                                raw = wrk_pool.tile(
                                    [128, CH], BF16, tag=f"raw{kind}"
                                )
                                nc.vector.tensor_copy(raw, acc)
                                sq = wrk_pool.tile([128, CH], BF16, tag="sq")
                                nc.vector.tensor_mul(sq, raw, raw)
                                ssq = psw.tile(
                                    [65, CH], F32, tag="w", name=f"ssq{kind}{rep_tag}"
                                )
                                nc.tensor.matmul(
                                    ssq, lhsT=onesMb, rhs=sq, start=True, stop=True
                                )
                                raws[kind] = raw
                                ssqs[kind] = ssq
                            else:
                                vtmp = wrk_pool.tile([128, CH], BF16, tag="vtmp")
                                nc.vector.tensor_copy(vtmp, acc)
                                vn = psw.tile(
                                    [128, CH], BF16, tag="w", name=f"vn{rep_tag}"
                                )
                                for nb in range(4):
                                    nc.tensor.transpose(
                                        vn[:, nb * 128 : (nb + 1) * 128],
                                        vtmp[:, nb * 128 : (nb + 1) * 128],
                                        identB,
                                    )
                                for nb in range(4):
                                    jb = cb * 4 + nb
                                    nc.vector.tensor_copy(
                                        v_all[:, jb, 0:64],
                                        vn[:, nb * 128 : nb * 128 + 64],
                                    )
                                    nc.vector.tensor_copy(
                                        v_all[:, jb, 65:129],
                                        vn[:, nb * 128 + 64 : (nb + 1) * 128],
                                    )
                        return raws, ssqs

                    def phase_N(cb, raws, ssqs):
                        n0 = cb * CH
                        # Ln, Ln then Exp, Exp: two table swaps per chunk
                        lg = wrk_pool.tile([65, 2, CH], F32, tag="lg")
                        nc.scalar.activation(lg[:, 0, :], ssqs["q"], AF.Ln)
                        nc.scalar.activation(lg[:, 1, :], ssqs["k"], AF.Ln)
                        inv = wrk_pool.tile([65, 2, CH], BF16, tag="inv")
                        nc.scalar.activation(inv[:, 0, :], lg[:, 0, :], AF.Exp, scale=-0.5)
                        nc.scalar.activation(inv[:, 1, :], lg[:, 1, :], AF.Exp, scale=-0.5)
                        for i, (kind, sel, dst) in enumerate(
                            (("q", selMb, qT), ("k", selMb8, kT))
                        ):
                            bc = psw.tile(
                                [128, CH], F32, tag="w", name=f"bc{kind}{rep_tag}"
                            )
                            nc.tensor.matmul(
                                bc, lhsT=sel, rhs=inv[:, i, :], start=True, stop=True
                            )
                            nc.vector.tensor_mul(
                                dst[:, n0 : n0 + CH], raws[kind], bc
                            )

                    def emit_tail(bi):
                        i0 = bi * CH
                        lps = psw.tile([128, CH], F32, tag="w", name=f"lps{rep_tag}")
                        nc.tensor.matmul(
                            lps,
                            lhsT=selMb,
                            rhs=l_row[:, i0 : i0 + CH],
                            start=True,
                            stop=True,
                        )
                        rbc = wrk_pool.tile([128, CH], F32, tag="rbc")
                        nc.vector.reciprocal(rbc, lps)
                        nc.vector.tensor_mul(
                            oT[:, i0 : i0 + CH], oT[:, i0 : i0 + CH], rbc
                        )
                        osb = osb_pool.tile([128, 4, D], F32, tag="osb")
                        for sub, ic in enumerate(range(4 * bi, 4 * (bi + 1))):
                            for nh in range(2):
                                op = psw.tile(
                                    [128, CH], F32, tag="w", name=f"op{rep_tag}"
                                )
                                nc.tensor.matmul(
                                    op,
                                    lhsT=oT[:, ic * 128 : (ic + 1) * 128],
                                    rhs=wo_sb[:, nh * CH : (nh + 1) * CH],
                                    start=True,
                                    stop=True,
                                )
                                nc.vector.tensor_copy(
                                    osb[:, sub, nh * CH : (nh + 1) * CH], op
                                )
                        nc.sync.dma_start(
                            out_d.rearrange("(b p) d -> p b d", p=128)[
                                :, bi * 4 : (bi + 1) * 4, :
                            ],
                            osb,
                        )

                    def phase_A(bi):
                        i0 = bi * CH
                        njb = 4 * (bi + 1)
                        ot = psot.tile([65, 2, CH], F32, tag="ot")
                        for jb in range(njb):
                            first = jb == 0
                            last = jb == njb - 1
                            # diagonal blocks (t>0) only cover i >= jb*128:
                            # shrink the i-window instead of masking it all
                            t = jb - 4 * bi
                            ioff = 128 * t if t > 0 else 0
                            w = CH - ioff
                            st = psst.tile(
                                [128, 2, CH], F32, tag="st", name=f"st{rep_tag}"
                            )
                            for h in range(2):
                                nc.tensor.matmul(
                                    st[:, h, 0:w],
                                    lhsT=kT[
                                        64 * h : 64 * (h + 1),
                                        jb * 128 : (jb + 1) * 128,
                                    ],
                                    rhs=qT[
                                        64 * h : 64 * (h + 1),
                                        i0 + ioff : i0 + CH,
                                    ],
                                    start=True,
                                    stop=True,
                                    tile_position=(64 * h, 0),
                                )
                            p = p_pool.tile([128, 2, CH], BF16, tag="p")
                            nc.scalar.activation(p[:, :, 0:w], st[:, :, 0:w], AF.Exp)
                            if t >= 0:
                                # keep where i-offset f >= partition j-offset
                                nc.gpsimd.affine_select(
                                    out=p[:, :, 0:w],
                                    in_=p[:, :, 0:w],
                                    compare_op=mybir.AluOpType.is_ge,
                                    fill=0.0,
                                    base=0,
                                    channel_multiplier=-1,
                                    pattern=[[0, 2], [1, w]],
                                )
                            nc.tensor.matmul(
                                ot[:, 0, ioff:CH],
                                lhsT=v_all[:, jb, 0:65],
                                rhs=p[:, 0, 0:w],
                                start=first,
                                stop=last,
                            )
                            nc.tensor.matmul(
                                ot[:, 1, ioff:CH],
                                lhsT=v_all[:, jb, 64:129],
                                rhs=p[:, 1, 0:w],
                                start=first,
                                stop=last,
                            )
                        stg = []
                        for h in range(2):
                            s = wrk_pool.tile(
                                [65, CH], BF16, tag="ostg", name=f"ostg{h}{rep_tag}"
                            )
                            nc.vector.tensor_copy(s, ot[:, h, :])
                            stg.append(s)
                        nc.sync.dma_start(oT[0:64, i0 : i0 + CH], stg[0][0:64, :])
                        nc.sync.dma_start(oT[64:128, i0 : i0 + CH], stg[1][1:65, :])
                        nc.sync.dma_start(l_row[0:1, i0 : i0 + CH], stg[0][64:65, :])
                        nc.sync.dma_start(l_row[64:65, i0 : i0 + CH], stg[1][0:1, :])

                    for cb in range(NCHUNK):
                        raws, ssqs = phase_P(cb)
                        phase_N(cb, raws, ssqs)
                        if cb >= 1:
                            phase_A(cb - 1)
                            if cb >= 2:
                                emit_tail(cb - 2)
                    emit_tail(NCHUNK - 2)
                    phase_A(NCHUNK - 1)
                    emit_tail(NCHUNK - 1)

            for _r in range(reps):
                body("" if _r == 0 else f"r{_r}")
    nc.compile()
    return nc


def _in_maps(x, Wq, Wk, Wv, Wo):
    maps = []
    for c in range(NCORES):
        cs = slice(c * C, (c + 1) * C)
        maps.append(
            {
                "x": x,
                "wq": np.ascontiguousarray(Wq[:, cs]),
                "wk": np.ascontiguousarray(Wk[:, cs]),
                "wv": np.ascontiguousarray(Wv[:, cs]),
                "wo": np.ascontiguousarray(Wo[cs, :]),
            }
        )
    return maps


_CACHE = {}


def _run_cached(in_maps):
    """Compile once per process; later kernel() calls only dispatch."""
    import jax
    from jax.experimental.shard_map import shard_map
    from jax.sharding import Mesh, PartitionSpec

    from concourse import bass2jax

    if "fn" not in _CACHE:
        bass2jax.install_neuronx_cc_hook()
        nc = build_nc()
        partition_name = (
            nc.partition_id_tensor.name if nc.partition_id_tensor else None
        )
        in_names, out_names, out_avals, zero_outs = [], [], [], []
        for alloc in nc.m.functions[0].allocations:
            if not isinstance(alloc, mybir.MemoryLocationSet):
                continue
            name = alloc.memorylocations[0].name
            if alloc.kind == "ExternalInput":
                if name != partition_name:
                    in_names.append(name)
            elif alloc.kind == "ExternalOutput":
                shape = list(alloc.tensor_shape)
                npdt = mybir.dt.np(alloc.dtype)
                out_names.append(name)
                out_avals.append(jax.core.ShapedArray(shape, npdt))
                zero_outs.append(np.zeros(shape, npdt))
        n_params = len(in_names)
        n_outs = len(out_avals)
        all_names = in_names + out_names
        if partition_name is not None:
            all_names.append(partition_name)
        donate = tuple(range(n_params, n_params + n_outs))

        def _body(*args):
            operands = list(args)
            if partition_name is not None:
                operands.append(bass2jax.partition_id_tensor())
            outs = bass2jax._bass_exec_p.bind(
                *operands,
                out_avals=tuple(out_avals),
                in_names=tuple(all_names),
                out_names=tuple(out_names),
                lowering_input_output_aliases=(),
                sim_require_finite=True,
                sim_require_nnan=True,
                nc=nc,
            )
            return tuple(outs)

        devices = jax.devices()[:NCORES]
        mesh = Mesh(np.asarray(devices), ("core",))
        fn = jax.jit(
            shard_map(
                _body,
                mesh=mesh,
                in_specs=(PartitionSpec("core"),) * (n_params + n_outs),
                out_specs=(PartitionSpec("core"),) * n_outs,
                check_rep=False,
            ),
            donate_argnums=donate,
            keep_unused=True,
        )
        _CACHE.update(
            fn=fn,
            in_names=in_names,
            out_names=out_names,
            zero_outs=zero_outs,
            mesh=mesh,
        )

    fn = _CACHE["fn"]
    mesh = _CACHE["mesh"]
    import jax as _jax
    from jax.sharding import NamedSharding, PartitionSpec as _P

    sh = NamedSharding(mesh, _P("core"))
    per_core = [
        [np.asarray(m[nm]) for nm in _CACHE["in_names"]] for m in in_maps
    ]
    concat_in = [
        np.concatenate([per_core[c][i] for c in range(NCORES)], axis=0)
        for i in range(len(_CACHE["in_names"]))
    ]
    dev_in = [_jax.device_put(a, sh) for a in concat_in]
    dz = [
        _jax.device_put(
            np.zeros((NCORES * z.shape[0], *z.shape[1:]), z.dtype), sh
        )
        for z in _CACHE["zero_outs"]
    ]
    outs = fn(*dev_in, *dz)
    res = []
    for c in range(NCORES):
        m = {}
        for i, nm in enumerate(_CACHE["out_names"]):
            rows = _CACHE["zero_outs"][i].shape[0]
            m[nm] = np.asarray(outs[i][c * rows : (c + 1) * rows])
        res.append(m)
    return res


def kernel(x, Wq, Wk, Wv, Wo):
    global LAST_EXEC_NS
    x = np.ascontiguousarray(np.asarray(x, dtype=np.float32).reshape(N, D))
    Wq = np.asarray(Wq, dtype=np.float32)
    Wk = np.asarray(Wk, dtype=np.float32)
    Wv = np.asarray(Wv, dtype=np.float32)
    Wo = np.asarray(Wo, dtype=np.float32)

    in_maps = _in_maps(x, Wq, Wk, Wv, Wo)
    try:
        results = _run_cached(in_maps)
    except Exception:
        nc = build_nc()
        res = run_bass_kernel_spmd(nc, in_maps, core_ids=list(range(NCORES)))
        LAST_EXEC_NS = getattr(res, "exec_time_ns", None)
        results = res.results

    out = np.zeros((N, D), dtype=np.float32)
    for c in range(NCORES):
        out += results[c]["out"]
    return out.reshape(1, N, D)
